# revision 43
# baseline (speedup 1.0000x reference)
"""Trainium2 SPMD kernel for nn_Attentionlayer_9208409883387.

Mathematical simplification: the reference computes
    h   = x @ W
    att = softmax(mask(leaky_relu(s1+s2), adj), axis=3)
    res = leaky_relu(h * sum_j att[..., j])
The row-sum of a softmax along its normalization axis is identically 1
(every row has >=1 unmasked entry: P[all-zero adj row] ~ 2^-1024), so
    res = leaky_relu(x @ W)
exactly, up to fp32 rounding of the softmax row-sum.

Strategy: data-parallel over the 48*1024 = 49152 rows, 6144 rows/core.
Each core's shard is laid out host-side with f_in on partitions
(rows[0:3072].T on partitions 0:64, rows[3072:6144].T on 64:128) so the
PE can consume it directly as the moving operand.  W is replicated as a
block-diagonal W (+) W [128,128] stationary operand.

I/O runs in bfloat16 both directions (halves the HBM traffic vs fp32;
measured rel-l2 ~3e-3, well under the 2e-2 gate).  Each input transfer
is its own DRAM tensor so the HBM read is fully sequential (strided
2KB-row reads measured ~150-250GB/s vs ~370-400GB/s sequential).

Pipeline: 4 chunks (512/1024/512/1024 cols), built WITHOUT nc.Block()
— its entry barrier stalls every engine on GpSimd's const MEMSETs and
its exit barrier duplicates the NRT postamble's own rendezvous; all
cross-engine ordering is carried explicitly by semaphores (worth
~0.6-0.8us).  Input triggers issue first, split across both HWDGE
rings (SP: W+c0, c2; ACT ring: c1, c3) so triggers, data, and
completion receipts overlap.  Matmuls (PSUM fp32) chain on arrival,
preceded by garbage warm-up matmuls that hold the PE's HAM activity
window open (the PE clock-gate defaults to 1.2GHz and only reaches
2.4GHz after ~3.4us of continuous busy).  The DVE computes chunk 2 as
max(x, 0.01x) (mul+max pair) in parallel with ACT, which applies Lrelu
to chunks 0, 1, 3 (guarded by init_sem against the const-MEMSET race).
Outputs stream as five ready-gated transfers: chunks 0-2 and half of
chunk 3 from the SP ring, the other half of chunk 3 from the ACT
engine's own ring so the final two transfers' data and receipts
overlap.

Measured on 8 cores: ~23.6us mean / ~24.1us worst-core on a clean run
(from the 29.8us fp32 single-stream baseline; HBM-contention noise can
push outlier cores to ~26us), of which ~13.7us is the fixed
NRT-injected NEFF preamble/postamble (the postamble alone zeroes the
full 253-semaphore file, ~7.3us) and the rest is the latency-chained
body: trigger ~0.7 + first-byte ~0.6 + in-data ~2.4 + in-receipt ~1.4
+ mm/act tail ~1.7 + out trigger/first-byte ~1.3 + out-data ~0.4 +
out-receipt ~1.1.
"""

import numpy as np

B, T, N, F = 4, 12, 1024, 64
N_CORES = 8
ROWS = B * T * N              # 49152
RPC = ROWS // N_CORES         # 6144 rows per core
HALF = RPC // 2               # 3072 packed columns per core

_PROGRAM = None

# compute chunks in packed-column space
CHUNKS = [(0, 512), (512, 1536), (1536, 2048), (2048, 3072)]
# The PE's HAM clock gate defaults to K=4/8 (1.2GHz) and only opens to
# 2.4GHz after ~3.4us of continuous busy (one full 4096-cycle activity
# window).  Warm matmuls bridge the gap from block start to the first
# input's arrival (~3.4us) so the real matmul chain runs at 2.4GHz.
N_WARM_MM = 16                # post-barrier [128,128] warms, ~107ns each cold


def _build_program_raw():
    import concourse.bass as bass
    import concourse.mybir as mybir
    from contextlib import ExitStack

    f32 = mybir.dt.float32
    bf16 = mybir.dt.bfloat16
    nc = bass.Bass("TRN2")
    # One DRAM tensor per input transfer -> sequential HBM reads.
    # xp0 carries the 128-col block-diag W ahead of chunk 0.
    xps = [
        nc.declare_dram_parameter(
            f"xp{i}", [128, (hi - lo) + (128 if i == 0 else 0)], bf16, isOutput=False
        )
        for i, (lo, hi) in enumerate(CHUNKS)
    ]
    yp = nc.declare_dram_parameter("ypack", [128, HALF], bf16, isOutput=True)

    with ExitStack() as ctx:
        x_sb = ctx.enter_context(nc.sbuf_tensor("x_sb", [128, 128 + HALF], bf16))
        y_sb = ctx.enter_context(nc.sbuf_tensor("y_sb", [128, HALF], bf16))
        tmp = ctx.enter_context(nc.sbuf_tensor("tmp", [128, 1024], f32))
        warm = ctx.enter_context(nc.sbuf_tensor("warm", [1, 4], f32))
        # banks 0-5 for real chunks, bank 6 as warm-up scratch
        ps = ctx.enter_context(nc.psum_tensor("ps", [128, HALF + 512], f32))
        din = [ctx.enter_context(nc.semaphore(f"din{i}")) for i in range(4)]
        pe_sem = ctx.enter_context(nc.semaphore("pe_sem"))
        act_sem = ctx.enter_context(nc.semaphore("act_sem"))
        dve_sem = ctx.enter_context(nc.semaphore("dve_sem"))
        dma_out = ctx.enter_context(nc.semaphore("dma_out"))

        init_sem = ctx.enter_context(nc.semaphore("init_sem"))

        # No nc.Block(): its entry barrier stalls every engine on GpSimd's
        # const MEMSETs (~0.7us) and its exit barrier is redundant with the
        # NRT postamble's own rendezvous.  All cross-engine ordering is
        # carried by the semaphores below; the one implicit dependency (ACT
        # reads the const bias that GpSimd memsets) is guarded by init_sem.
        # Input DMA triggers go first so data streams during engine init.
        # The scalar ring's triggers are intentionally held behind init_sem
        # (~0.6us): T0's data then streams uncontended at full SDMA rate,
        # so din0 — the gate for the whole compute chain — fires earlier.
        nc.sync.dma_start(out=x_sb[:, 0:640], in_=xps[0][:, :]).then_inc(din[0], 16)
        nc.sync.dma_start(out=x_sb[:, 1664:2176], in_=xps[2][:, :]).then_inc(
            din[2], 16
        )
        # HAM warm-up: garbage matmuls into scratch bank 6 keep the PE
        # continuously busy while the input transfers are in flight.
        w_ap = x_sb[:, 0:128]
        for _ in range(10 + N_WARM_MM):
            nc.tensor.matmul(
                ps[:, HALF : HALF + 128],
                w_ap,
                x_sb[:, 128:256],
                start=True,
                stop=True,
            )

        # GpSimd: signal that walrus's const MEMSETs (emitted at the head of
        # the GpSimd stream) have retired; ACT waits on this before its
        # first const-reading activation.
        nc.gpsimd.memset(warm[:, :], 0.0).then_inc(init_sem, 1)

        # --- SP engine: ready-gated outputs, then the final completion wait
        nc.sync.wait_ge(act_sem, 1)
        nc.sync.dma_start(out=yp[:, 0:512], in_=y_sb[:, 0:512]).then_inc(dma_out, 16)
        nc.sync.wait_ge(act_sem, 2)
        nc.sync.dma_start(out=yp[:, 512:1536], in_=y_sb[:, 512:1536]).then_inc(
            dma_out, 16
        )
        nc.sync.wait_ge(dve_sem, 1)
        nc.sync.dma_start(out=yp[:, 1536:2048], in_=y_sb[:, 1536:2048]).then_inc(
            dma_out, 16
        )
        # second half of chunk 3 in parallel with the ACT-ring half
        nc.sync.wait_ge(act_sem, 3)
        nc.sync.dma_start(out=yp[:, 2560:3072], in_=y_sb[:, 2560:3072]).then_inc(
            dma_out, 16
        )
        nc.sync.wait_ge(dma_out, 80)

        # --- PE: real matmul chain, gated per chunk on input arrival
        for i, (lo, hi) in enumerate(CHUNKS):
            nc.tensor.wait_ge(din[i], 16)
            for mlo in range(lo, hi, 512):
                nc.tensor.matmul(
                    ps[:, mlo : mlo + 512],
                    w_ap,
                    x_sb[:, 128 + mlo : 128 + mlo + 512],
                    start=True,
                    stop=True,
                ).then_inc(pe_sem, 1)

        # --- ACT: init-guard, its two input transfers (deliberately after
        # the SP ring's so T0 is uncontended), then the Lrelu table warm-up
        # (walrus's lazy ACT_TABLE_LOAD ~1.3us runs during the DMA stream)
        # and chunks 0, 1, 3.
        nc.scalar.wait_ge(init_sem, 1)
        nc.scalar.dma_start(out=x_sb[:, 640:1664], in_=xps[1][:, :]).then_inc(
            din[1], 16
        )
        nc.scalar.dma_start(out=x_sb[:, 2176:3200], in_=xps[3][:, :]).then_inc(
            din[3], 16
        )
        nc.scalar.activation(
            warm[:, :], warm[:, :],
            mybir.ActivationFunctionType.Lrelu, alpha=0.01,
        )
        # mm counts per chunk [1,2,1,2]
        for ci, need in ((0, 1), (1, 3), (3, 6)):
            lo, hi = CHUNKS[ci]
            nc.scalar.wait_ge(pe_sem, need)
            nc.scalar.activation(
                y_sb[:, lo:hi],
                ps[:, lo:hi],
                mybir.ActivationFunctionType.Lrelu,
                alpha=0.01,
            ).then_inc(act_sem, 1)
        # first half of chunk 3's output straight from the ACT engine (its
        # HWDGE ring is idle by now); the SP ring carries the other half
        # concurrently.  The wait_ge on act_sem (which the activation
        # increments at completion) is required: the sequencer pipelines the
        # DMA trigger into the ACTIVATE, so without it the SDMA can read
        # y_sb before the activation's writes retire.
        nc.scalar.wait_ge(act_sem, 3)
        nc.scalar.dma_start(out=yp[:, 2048:2560], in_=y_sb[:, 2048:2560]).then_inc(
            dma_out, 16
        )

        # --- DVE: leaky_relu = max(x, 0.01x); DVE can read only one PSUM
        # operand per instruction, so stage 0.01x through SBUF.
        lo, hi = CHUNKS[2]
        nc.vector.wait_ge(pe_sem, 4)
        nc.vector.tensor_scalar_mul(tmp[:, 0:512], ps[:, lo:hi], 0.01)
        nc.vector.tensor_max(
            y_sb[:, lo:hi], ps[:, lo:hi], tmp[:, 0:512]
        ).then_inc(dve_sem, 1)

    nc.finalize()
    return nc


_build_program = _build_program_raw


def _get_program():
    global _PROGRAM
    if _PROGRAM is None:
        _PROGRAM = _build_program()
    return _PROGRAM


def _make_in_maps(x, W):
    import ml_dtypes

    bf16 = ml_dtypes.bfloat16
    xr = np.ascontiguousarray(x, dtype=np.float32).reshape(N_CORES, RPC, F)
    wpack = np.zeros((128, 128), np.float32)
    wpack[0:64, 0:64] = W
    wpack[64:128, 64:128] = W
    wpack16 = wpack.astype(bf16)
    in_maps = []
    for c in range(N_CORES):
        xpack = np.empty((128, HALF), bf16)
        xpack[0:64] = xr[c, 0:HALF].T.astype(bf16)
        xpack[64:128] = xr[c, HALF:].T.astype(bf16)
        m = {}
        for i, (lo, hi) in enumerate(CHUNKS):
            if i == 0:
                m["xp0"] = np.ascontiguousarray(
                    np.concatenate([wpack16, xpack[:, lo:hi]], axis=1)
                )
            else:
                m[f"xp{i}"] = np.ascontiguousarray(xpack[:, lo:hi])
        in_maps.append(m)
    return in_maps


def run_spmd(x, W, **spmd_kwargs):
    """Run the Bass program on 8 cores; returns (y_full, BassKernelResults)."""
    from concourse.bass_utils import run_bass_kernel_spmd

    in_maps = _make_in_maps(x, W)
    res = run_bass_kernel_spmd(
        _get_program(), in_maps, list(range(N_CORES)), **spmd_kwargs
    )
    y = np.empty((N_CORES, RPC, F), np.float32)
    for c in range(N_CORES):
        ypack = np.asarray(res.results[c]["ypack"]).astype(np.float32)
        y[c, 0:HALF] = ypack[0:64].T
        y[c, HALF:] = ypack[64:128].T
    return y.reshape(B, T, N, F), res


def kernel(x, adj, W, a):
    # adj and a are mathematically dead (softmax row-sum == 1); see module doc.
    y, _ = run_spmd(np.asarray(x), np.asarray(W, dtype=np.float32))
    return y
